# revision 2
# baseline (speedup 1.0000x reference)
"""VQ codebook reconstruction kernel for Trainium2 (8 NeuronCores, SPMD).

Reference computation (per pixel feature vector f in R^C):
    weights = (codebook @ f) / ||codebook_rows||^2      # [N]
    recon   = codebook.T @ weights                      # [C]

This collapses to a single fixed matrix applied per pixel:
    recon = M @ f,   M = codebook.T @ diag(1/||c_n||^2) @ codebook   # [C, C]

M is tiny ([256,256]) and is formed on the host in float64; the device
kernel applies M to all B*H*W = 131072 pixel vectors, sharded
data-parallel over (B, H) across 8 cores.

The kernel is DMA-bound: 16.78 MB fp32 read + 8.39 MB fp16 write per
core against a ~420 GB/s 16-engine SDMA pool => ~60 us of pure DMA.
Schedule: ALL input-slab reads are enqueued up-front on the two HWDGE
rings (sync + scalar) so the SDMA pool never idles; the SDMA engines
round-robin between rings at packet granularity, so output writes
(gpsimd SWDGE ring) interleave at fair share as they become ready.
Slab sizes taper (512..2048..512) so the first matmul starts ~9.5 us
in and the final read->matmul->cast->write chain is short. PSUM->SBUF
fp32->fp16 casts are split across the vector (mb=0) and scalar ACT
(mb=1) engines, 2048 cols per instruction to amortize the cayman
SBUF-op errata bubble. Matmuls use float32r (1 cycle/row PE rate).
"""

import numpy as np

B, C, H, W = 4, 256, 128, 256
N_CORES = 8
SPLIT_H = 2            # 8 shards = B(4) x H-halves(2)
SH = H // SPLIT_H      # 64 rows of H per shard
P_SHARD = SH * W       # 16384 pixels per core

# Tapered slab sizes: small head (early compute start), small tail
# (short serial drain chain), 2048 bulk (big DMAs, wide casts).
SLABS = [512, 512, 1024, 2048, 2048, 2048, 2048, 2048, 2048, 1024, 512, 512]
assert sum(SLABS) == P_SHARD

_NC_CACHE = {}


def _build_nc():
    if "nc" in _NC_CACHE:
        return _NC_CACHE["nc"]

    import concourse.bass as bass
    import concourse.tile as tile
    from concourse import bacc, mybir

    f32 = mybir.dt.float32
    f16 = mybir.dt.float16
    f32r = mybir.dt.float32r

    nc = bacc.Bacc()
    feat = nc.dram_tensor("feat", [C, P_SHARD], f32r, kind="ExternalInput")
    # mmat host layout: [128, 2, 256] = (partition, K-half, col)
    mmat = nc.dram_tensor("mmat", [128, 2, C], f32r, kind="ExternalInput")
    # fp16 output halves write traffic; host upcasts to fp32 (exact).
    out = nc.dram_tensor("out", [C, P_SHARD], f16, kind="ExternalOutput")

    # feat rows are (a*128 + p); view as [p, a, n] so one DMA per slab
    # pulls both K-halves.
    feat3 = feat.rearrange("(a k) n -> k a n", a=2)
    out3 = out.rearrange("(a p) n -> p a n", a=2)

    MAXS = max(SLABS)

    with tile.TileContext(nc) as tc:
        with (
            tc.tile_pool(name="mpool", bufs=1) as mpool,
            tc.tile_pool(name="rhs", bufs=1) as rhs_pool,
            tc.tile_pool(name="opool", bufs=4) as opool,
            tc.tile_pool(name="psum", bufs=1, space="PSUM") as psum_pool,
        ):
            # M on the gpsimd ring (idle until writes start); reads lead
            # the sync/scalar HWDGE rings so slab 0 lands ASAP.
            mt = mpool.tile([128, 2, C], f32r, tag="m")
            nc.gpsimd.dma_start(mt[:], mmat[:, :, :])

            # Enqueue ALL slab reads up-front, alternating HWDGE rings.
            rts = []
            off = 0
            for j, S in enumerate(SLABS):
                rt = rhs_pool.tile([128, 2, S], f32r, tag=f"r{j}", name=f"rt{j}")
                eng = nc.sync if (j % 2 == 0) else nc.scalar
                eng.dma_start(rt[:], feat3[:, :, off:off + S])
                rts.append(rt)
                off += S

            off = 0
            for j, S in enumerate(SLABS):
                rt = rts[j]
                NSUB = S // 512
                ot = opool.tile([128, 2, S], f16, tag="o", name=f"ot{j}")
                ps = [
                    psum_pool.tile([128, MAXS], f32, tag=f"ps{mb}",
                                   name=f"ps{j}_{mb}")
                    for mb in range(2)
                ]
                for mb in range(2):
                    for kb in range(2):
                        for n in range(NSUB):
                            nc.tensor.matmul(
                                ps[mb][:, n * 512:(n + 1) * 512],
                                mt[:, kb, mb * 128:(mb + 1) * 128],
                                rt[:, kb, n * 512:(n + 1) * 512],
                                start=(kb == 0),
                                stop=(kb == 1),
                            )
                    # Cast PSUM fp32 -> SBUF fp16; split across engines.
                    if mb == 0:
                        nc.vector.tensor_copy(ot[:, 0, :], ps[0][:, :S])
                    else:
                        nc.scalar.copy(ot[:, 1, :], ps[1][:, :S])
                nc.gpsimd.dma_start(out3[:, :, off:off + S], ot[:])
                off += S

    nc.compile()
    _NC_CACHE["nc"] = nc
    return nc


def _host_prep(feature, codebook):
    cb = codebook.astype(np.float64)
    norm = np.sum(cb * cb, axis=1)
    m = ((cb / norm[:, None]).T @ cb).astype(np.float32)
    # [256, 256] -> [128, 2, 256]: (k-half a, partition p) -> row a*128+p
    m3 = np.ascontiguousarray(m.reshape(2, 128, C).transpose(1, 0, 2))

    in_maps = []
    for i in range(N_CORES):
        b, hs = i // SPLIT_H, (i % SPLIT_H) * SH
        shard = np.ascontiguousarray(
            feature[b, :, hs:hs + SH, :].reshape(C, P_SHARD)
        )
        in_maps.append({"feat": shard, "mmat": m3})
    return in_maps


def _gather(results):
    out = np.empty((B, C, H, W), dtype=np.float32)
    for i in range(N_CORES):
        b, hs = i // SPLIT_H, (i % SPLIT_H) * SH
        out[b, :, hs:hs + SH, :] = results[i]["out"].reshape(C, SH, W).astype(np.float32)
    return out


def run(feature, codebook, **spmd_kwargs):
    from concourse.bass_utils import run_bass_kernel_spmd

    nc = _build_nc()
    in_maps = _host_prep(np.asarray(feature), np.asarray(codebook))
    res = run_bass_kernel_spmd(nc, in_maps, list(range(N_CORES)), **spmd_kwargs)
    return _gather(res.results), res


def kernel(feature, codebook):
    out, _ = run(feature, codebook)
    return out


# revision 5
# speedup vs baseline: 1.2591x; 1.2591x over previous
"""VQ codebook reconstruction kernel for Trainium2 (8 NeuronCores, SPMD).

Reference computation (per pixel feature vector f in R^C):
    weights = (codebook @ f) / ||codebook_rows||^2      # [N]
    recon   = codebook.T @ weights                      # [C]

This collapses to a single fixed matrix applied per pixel:
    recon = M @ f,   M = codebook.T @ diag(1/||c_n||^2) @ codebook   # [C, C]

M is tiny ([256,256]) and is formed on the host in float64; the device
kernel applies M to all B*H*W = 131072 pixel vectors, sharded
data-parallel over (B, H) across 8 cores.

The kernel is DMA-bound: 16.78 MB fp32 read + 8.39 MB fp16 write per
core against a ~420 GB/s 16-engine SDMA pool => ~60 us of pure DMA.
Schedule highlights:
  * M goes FIRST on the sync HWDGE ring, then ALL slab reads are
    enqueued up-front alternating sync/scalar HWDGE rings; the SDMA
    engines round-robin rings at packet granularity so the pool never
    idles and writes get bandwidth as they appear.
  * Bulk writes go on the gpsimd(Pool) SWDGE ring (~210 GB/s solo cap,
    enough for the ~150 GB/s steady write stream); the last slabs'
    writes move to the sync/scalar HWDGE rings once their read FIFOs
    drain, so the final write drain spans three rings.
  * Slab sizes taper (512..2048..512): compute starts ~10 us in and
    the final read->matmul->cast->write chain is short.
  * Matmuls keep the kb-adjacent per-region order (the PE runs it at
    mid p-state 475 ns/512col; grouping by stationary measured slower).
  * PSUM->SBUF fp32->fp16 casts are split: vector(DVE) takes row-half
    0, scalar(ACT, idle otherwise, fast PSUM port) takes row-half 1,
    1024 cols per instruction to amortize the cayman SBUF-op errata.
"""

import numpy as np

B, C, H, W = 4, 256, 128, 256
N_CORES = 8
SPLIT_H = 2            # 8 shards = B(4) x H-halves(2)
SH = H // SPLIT_H      # 64 rows of H per shard
P_SHARD = SH * W       # 16384 pixels per core

SLABS = [512, 512, 1024, 2048, 2048, 2048, 2048, 2048, 2048, 1024, 512, 512]
assert sum(SLABS) == P_SHARD

_NC_CACHE = {}


def _build_nc():
    if "nc" in _NC_CACHE:
        return _NC_CACHE["nc"]

    import concourse.bass as bass
    import concourse.tile as tile
    from concourse import bacc, mybir

    f32 = mybir.dt.float32
    f16 = mybir.dt.float16
    f32r = mybir.dt.float32r

    nc = bacc.Bacc()
    feat = nc.dram_tensor("feat", [C, P_SHARD], f32r, kind="ExternalInput")
    # mmat host layout: [128, 2, 256] = (partition, K-half, col)
    mmat = nc.dram_tensor("mmat", [128, 2, C], f32r, kind="ExternalInput")
    # fp16 output halves write traffic; host upcasts to fp32 (exact).
    out = nc.dram_tensor("out", [C, P_SHARD], f16, kind="ExternalOutput")

    feat3 = feat.rearrange("(a k) n -> k a n", a=2)
    out3 = out.rearrange("(a p) n -> p a n", a=2)

    with tile.TileContext(nc) as tc:
        with (
            tc.tile_pool(name="mpool", bufs=1) as mpool,
            tc.tile_pool(name="rhs", bufs=1) as rhs_pool,
            tc.tile_pool(name="opool", bufs=4) as opool,
            tc.tile_pool(name="psum", bufs=2, space="PSUM") as psum_pool,
        ):
            # M first on the fast sync HWDGE ring (ring is FIFO, so it
            # lands before slab 0 and matmuls can start ~10us in).
            mt = mpool.tile([128, 2, C], f32r, tag="m")
            nc.sync.dma_start(mt[:], mmat[:, :, :])

            # Enqueue ALL slab reads up-front, alternating HWDGE rings.
            rts = []
            off = 0
            for j, S in enumerate(SLABS):
                rt = rhs_pool.tile([128, 2, S], f32r, tag=f"r{j}", name=f"rt{j}")
                eng = nc.sync if (j % 2 == 0) else nc.scalar
                eng.dma_start(rt[:], feat3[:, :, off:off + S])
                rts.append(rt)
                off += S

            off = 0
            for j, S in enumerate(SLABS):
                rt = rts[j]
                ot = opool.tile([128, 2, S], f16, tag="o", name=f"ot{j}")
                NG = (S + 1023) // 1024           # 1024-col cast groups
                for g in range(NG):
                    GW = min(1024, S - g * 1024)  # group width (512 or 1024)
                    ps = [
                        psum_pool.tile([128, 1024], f32, tag=f"ps{mb}",
                                       name=f"ps{j}_{g}_{mb}")
                        for mb in range(2)
                    ]
                    for n2 in range(GW // 512):
                        n = g * 2 + n2            # 512-col region in slab
                        for mb in range(2):
                            for kb in range(2):
                                nc.tensor.matmul(
                                    ps[mb][:, n2 * 512:(n2 + 1) * 512],
                                    mt[:, kb, mb * 128:(mb + 1) * 128],
                                    rt[:, kb, n * 512:(n + 1) * 512],
                                    start=(kb == 0),
                                    stop=(kb == 1),
                                )
                    lo, hi = g * 1024, g * 1024 + GW
                    nc.vector.tensor_copy(ot[:, 0, lo:hi], ps[0][:, :GW])
                    nc.scalar.copy(ot[:, 1, lo:hi], ps[1][:, :GW])
                # Bulk writes ride the gpsimd SWDGE ring; late-slab writes
                # hop to the sync/scalar HWDGE rings (their FIFO read
                # queues have drained by then), spreading write bandwidth
                # across three rings for the drain phase.
                if j <= 6:
                    weng = nc.gpsimd
                elif j % 2 == 0:
                    weng = nc.sync
                else:
                    weng = nc.scalar
                weng.dma_start(out3[:, :, off:off + S], ot[:])
                off += S

    nc.compile()
    _NC_CACHE["nc"] = nc
    return nc


def _host_prep(feature, codebook):
    cb = codebook.astype(np.float64)
    norm = np.sum(cb * cb, axis=1)
    m = ((cb / norm[:, None]).T @ cb).astype(np.float32)
    # [256, 256] -> [128, 2, 256]: row a*128+p -> (p, a)
    m3 = np.ascontiguousarray(m.reshape(2, 128, C).transpose(1, 0, 2))

    in_maps = []
    for i in range(N_CORES):
        b, hs = i // SPLIT_H, (i % SPLIT_H) * SH
        shard = np.ascontiguousarray(
            feature[b, :, hs:hs + SH, :].reshape(C, P_SHARD)
        )
        in_maps.append({"feat": shard, "mmat": m3})
    return in_maps


def _gather(results):
    out = np.empty((B, C, H, W), dtype=np.float32)
    for i in range(N_CORES):
        b, hs = i // SPLIT_H, (i % SPLIT_H) * SH
        out[b, :, hs:hs + SH, :] = results[i]["out"].reshape(C, SH, W).astype(np.float32)
    return out


def run(feature, codebook, **spmd_kwargs):
    from concourse.bass_utils import run_bass_kernel_spmd

    nc = _build_nc()
    in_maps = _host_prep(np.asarray(feature), np.asarray(codebook))
    res = run_bass_kernel_spmd(nc, in_maps, list(range(N_CORES)), **spmd_kwargs)
    return _gather(res.results), res


def kernel(feature, codebook):
    out, _ = run(feature, codebook)
    return out
